# revision 8
# baseline (speedup 1.0000x reference)
"""Mixed pooling (2x2, training mode) Trainium2 kernel — fp32 exact.

    out[b,ho,wo,c] = (1-k)*max(window) + k*mean(window),   k in {0,1}

Design (8 NeuronCores, pure batch data-parallel, 4 batches/core):
 - Per core, per (batch, w-block of 32 input columns): load x fp32 with
   partition dim = ho (exactly 128 output rows), free dim = (dh, w, c).
   All DMAs are SWDGE (gpsimd-issued) — HWDGE measured pathologically
   slow on this stack. k loads as raw int32.
 - VectorE computes both trees and the blend in fp32:
     mh = max(A,B); sh = A+B           (h-pool, contiguous halves)
     mx = max(mh_e, mh_o); sx = sh_e+sh_o   (w-pool, strided even/odd c-runs)
     km = 1-k; kq = 0.25*k             (tensor_scalar, int32 -> fp32)
     out = mx*km + sx*kq               (exact select: k is 0/1)
   The select form avoids the mean/max cancellation of
   out = mx + k*(mean-mx), so results match the fp32 reference to ~1e-7.
 - Raw bass (no Tile): this walrus build allows only ONE sync-wait per
   instruction, so all waits are standalone wait_ge ops. Every DMA gets
   its own (rotating) semaphore — cumulative counting on a shared sem is
   unsound under SDMA engine skew. DVE semaphore increments are kept to
   one per block: high-rate per-op then_inc measured ~450us each.
 - Measured single-core steady state: ~458 us/shard — the SBUF-fabric
   roofline (201 MB / 435 GB/s); with all 8 cores sharing HBM stacks the
   expected steady state is ~560 us (201 MB / 358 GB/s).

`nrep` repeats the whole pipeline in-program (benchmark use only).
"""
import numpy as np

import concourse.bass as bass
from concourse import mybir
from concourse.bass_utils import run_bass_kernel_spmd

B, H, W, C = 32, 256, 256, 128
NCORES = 8
BPC = B // NCORES          # batches per core
PH, Ho, Wo = 128, H // 2, W // 2
WBLK = 32                  # input w columns per block
WOB = WBLK // 2
NWB = W // WBLK
NBLK = BPC * NWB           # 32 blocks per core
NBUF = 2                   # double buffering

F_X = 2 * WBLK * C         # xt free elems
F_H = WBLK * C
F_O = WOB * C

f32 = mybir.dt.float32
i32 = mybir.dt.int32


def _build(nrep: int = 1):
    nc = bass.Bass("TRN2", debug=False, num_devices=NCORES)
    x = nc.dram_tensor("x", [BPC, H, W, C], f32, kind="ExternalInput").ap()
    k = nc.dram_tensor("k", [BPC, Ho, Wo, C], i32, kind="ExternalInput").ap()
    o = nc.dram_tensor("o", [BPC, Ho, Wo, C], f32, kind="ExternalOutput").ap()

    # per-block DRAM views; partition dim = ho (exactly 128)
    xv = x.rearrange("b (ho dh) (wb w) c -> b ho dh wb w c", dh=2, w=WBLK)
    kv = k.rearrange("b ho (wb wo) c -> b ho wb wo c", wo=WOB)
    ov = o.rearrange("b ho (wb wo) c -> b ho wb wo c", wo=WOB)

    blocks = [(b, wb) for b in range(BPC) for wb in range(NWB)]

    from contextlib import ExitStack

    with ExitStack() as ctx:
        def sb(name, fdim, dt=f32):
            return [
                ctx.enter_context(nc.sbuf_tensor(f"{name}{i}", [PH, fdim], dt))
                for i in range(NBUF)
            ]

        xt = sb("xt", F_X)
        kt = sb("kt", F_O, i32)
        mh = sb("mh", F_H)
        sh = sb("sh", F_H)
        mx = sb("mx", F_O)
        sx = sb("sx", F_O)

        # Rotating sem pools: index g%M, cumulative target per use. Spacing
        # M >= 2*NBUF guarantees the previous use completed before reuse.
        M = 8
        xsem = [ctx.enter_context(nc.semaphore(f"xs{i}")) for i in range(M)]
        ksem = [ctx.enter_context(nc.semaphore(f"ks{i}")) for i in range(M)]
        ssem = [ctx.enter_context(nc.semaphore(f"ss{i}")) for i in range(M)]
        cmp_sem = ctx.enter_context(nc.semaphore("cmp"))

        def sem_of(pool, g):
            return pool[g % M], g // M + 1

        block = ctx.enter_context(nc.Block())
        NG = nrep * NBLK

        @block.gpsimd
        def _(g: bass.BassEngine):
            for gi in range(NG):
                i = gi % NBLK
                b, wb = blocks[i]
                s = gi % NBUF
                if gi >= NBUF:
                    # buffer set s free once block gi-NBUF fully computed
                    g.wait_ge(cmp_sem, gi - NBUF + 1)
                sem, n = sem_of(xsem, gi)
                g.dma_start(out=xt[s][:], in_=xv[b, :, :, wb]).then_inc(sem, 16)
                sem, n = sem_of(ksem, gi)
                g.dma_start(out=kt[s][:], in_=kv[b, :, wb]).then_inc(sem, 16)
                if gi >= 1:
                    pg = gi - 1
                    pb, pwb = blocks[pg % NBLK]
                    g.wait_ge(cmp_sem, gi)
                    sem, n = sem_of(ssem, pg)
                    g.dma_start(
                        out=ov[pb, :, pwb], in_=mx[pg % NBUF][:]
                    ).then_inc(sem, 16)
            lg = NG - 1
            lb, lwb = blocks[lg % NBLK]
            g.wait_ge(cmp_sem, NG)
            sem, n = sem_of(ssem, lg)
            g.dma_start(out=ov[lb, :, lwb], in_=mx[lg % NBUF][:]).then_inc(sem, 16)
            # single SWDGE queue is FIFO per engine: last store done => all done
            g.wait_ge(sem, 16 * n)

        @block.vector
        def _(v: bass.BassEngine):
            for gi in range(NG):
                s = gi % NBUF
                A = xt[s][:, 0:F_H]
                Bs = xt[s][:, F_H:F_X]
                mhv = mh[s][:].rearrange("p (w c) -> p w c", c=C)
                shv = sh[s][:].rearrange("p (w c) -> p w c", c=C)
                km = mh[s][:, 0:F_O]       # mh reusable after the w-pool max
                kq = mh[s][:, F_O : 2 * F_O]

                sem, n = sem_of(xsem, gi)
                v.wait_ge(sem, 16 * n)
                v.tensor_max(out=mh[s][:], in0=A, in1=Bs)
                v.tensor_add(out=sh[s][:], in0=A, in1=Bs)
                if gi >= NBUF:
                    # mx[s] is the store source of block gi-NBUF
                    sem, n = sem_of(ssem, gi - NBUF)
                    v.wait_ge(sem, 16 * n)
                v.tensor_max(out=mx[s][:], in0=mhv[:, 0::2, :], in1=mhv[:, 1::2, :])
                v.tensor_add(out=sx[s][:], in0=shv[:, 0::2, :], in1=shv[:, 1::2, :])
                sem, n = sem_of(ksem, gi)
                v.wait_ge(sem, 16 * n)
                v.tensor_scalar(
                    out=km, in0=kt[s][:], scalar1=-1.0, scalar2=1.0,
                    op0=mybir.AluOpType.mult, op1=mybir.AluOpType.add,
                )
                v.tensor_scalar_mul(kq, kt[s][:], 0.25)
                v.tensor_mul(out=mx[s][:], in0=mx[s][:], in1=km)
                v.tensor_mul(out=sx[s][:], in0=sx[s][:], in1=kq)
                v.tensor_add(out=mx[s][:], in0=mx[s][:], in1=sx[s][:]).then_inc(
                    cmp_sem, 1
                )

    return nc


_NC = None


def kernel(x: np.ndarray, k: np.ndarray) -> np.ndarray:
    global _NC
    if _NC is None:
        _NC = _build()
    xs = x.reshape(NCORES, BPC, H, W, C)
    ks = k.reshape(NCORES, BPC, Ho, Wo, C)
    in_maps = [{"x": np.ascontiguousarray(xs[i]), "k": np.ascontiguousarray(ks[i])}
               for i in range(NCORES)]
    res = run_bass_kernel_spmd(_NC, in_maps, core_ids=list(range(NCORES)))
    out = np.concatenate([res.results[i]["o"] for i in range(NCORES)], axis=0)
    return out.reshape(B, Ho, Wo, C).astype(np.float32)
